# revision 1
# baseline (speedup 1.0000x reference)
"""Trainium2 Bass kernel for nn_MultiHeadClassifier (moe_routing).

Strategy: data-parallel over the N=32768 points axis across 8 NeuronCores
(4096 points/core). Weights replicated. BatchNorm batch statistics are
computed ANALYTICALLY from the feature Gram matrix C = F^T F (per-core
partial, AllReduce'd), so the big activation x1 = features @ W1 is
computed in a single fused pass:

  x1 (PE, bf16) -> BN+LeakyReLU (one ACT Lrelu op, per-channel scale/bias)
     -> per-category head matmul vs a 96-col expanded block-diagonal weight
     -> transpose -> masked log-softmax -> output columns 0..49.

The 96 expanded columns are laid out so columns 0..49 ARE the scattered
output columns (the 16 category segments partition [0,50) exactly) and
columns 50..95 hold the "overflow" logits (category c's logits j >=
seg_lens[c]) which participate in the softmax denominator only.
"""

import os
import sys
import functools
from contextlib import ExitStack

import numpy as np
import ml_dtypes

BF = ml_dtypes.bfloat16

for _p in ("/opt/trn_rl_repo", "/root/.axon_site/_ro/trn_rl_repo"):
    if os.path.isdir(_p) and _p not in sys.path:
        sys.path.insert(0, _p)

import concourse.bass as bass
import concourse.tile as tile
from concourse import bacc
from concourse import mybir
from concourse.bass_utils import run_bass_kernel_spmd
from concourse.masks import make_identity
from concourse.tile_rust import add_dep_helper

NCORES = 8
NPTS = 4096          # points per core
NCH = 4096           # C*F hidden channels
KF = 256             # input features
NCAT = 16
SEG = 6              # max segments per category
U = 96               # expanded feature columns (NCAT*SEG)
OUTW = 50
NB = 1024            # point block (two PSUM banks of fp32)
NPB = NPTS // NB     # 4
NHALF = NB // 512    # matmul N<=512 sub-blocks
MCH = NCH // 128     # 32 channel chunks
N_GLOBAL = NCORES * NPTS
BN_EPS = 1e-5
BIG = 300.0          # mask offset; exp(-BIG) == 0 in fp32
LEAK = 0.2

f32 = mybir.dt.float32
bf16 = mybir.dt.bfloat16
AF = mybir.ActivationFunctionType
ALU = mybir.AluOpType




class _Bacc(bacc.Bacc):
    """Prefer natural_log_exp_and_others (parametric_relu + exp + ln) so the
    main loop never swaps activation tables."""

    def insert_act_table_loads(self):
        import bass_rust as _br
        from concourse.hw_specs import get_activation_tables
        has_activation = any(
            isinstance(i, mybir.InstActivation)
            for b in self.main_func.blocks
            for i in b.instructions
        )
        if not has_activation:
            return
        # act_func_set_id is the POSITION in act_info.json's list, so keep
        # every entry in place; just empty the funcs of non-preferred sets so
        # the chooser can only pick these (no per-iteration table swapping).
        keep = ("natural_log_exp_and_others", "sqrt_and_others")
        tables = [
            (name, funcs if name in keep else set())
            for name, funcs in get_activation_tables(self.m.arch).items()
        ]
        _br.insert_act_table_loads(self, tables)


def _normalize(nc, dvw, x2, px, a_t, b_t, m):
    """x2 = LeakyReLU(a*x1 + b), split across ScalarE and VectorE to
    balance engine load (ACT path ~62%% of chunks)."""
    if m % 8 < 5:
        nc.scalar.activation(
            out=x2, in_=px, func=AF.Prelu,
            bias=b_t[:, m:m + 1], scale=a_t[:, m:m + 1], alpha=LEAK,
        )
    else:
        y = dvw.tile([128, px.shape[-1]], bf16, tag="y")
        nc.vector.tensor_scalar(
            out=y, in0=px, scalar1=a_t[:, m:m + 1],
            scalar2=b_t[:, m:m + 1], op0=ALU.mult, op1=ALU.add,
        )
        t02 = dvw.tile([128, px.shape[-1]], bf16, tag="t02")
        nc.vector.tensor_scalar_mul(out=t02, in0=y, scalar1=LEAK)
        nc.vector.tensor_tensor(out=x2, in0=y, in1=t02, op=ALU.max)


def build_program():
    nc = _Bacc()

    featT_d = nc.dram_tensor("featT", [128, 2, NPTS], bf16, kind="ExternalInput")
    fnat_d = nc.dram_tensor("fnat", [128, MCH, KF], bf16, kind="ExternalInput")
    w1_d = nc.dram_tensor("w1", [128, 2, NCH], bf16, kind="ExternalInput")
    wbig_d = nc.dram_tensor("wbig", [128, MCH, U], bf16, kind="ExternalInput")
    gam_d = nc.dram_tensor("gamma_t", [128, MCH], f32, kind="ExternalInput")
    bet_d = nc.dram_tensor("beta_t", [128, MCH], f32, kind="ExternalInput")
    catf_d = nc.dram_tensor("catf", [128, NPTS // 128], f32, kind="ExternalInput")
    catid_d = nc.dram_tensor("catid96", [1, U], f32, kind="ExternalInput")
    bias_d = nc.dram_tensor("bias96", [1, U], f32, kind="ExternalInput")
    out_d = nc.dram_tensor("out", [NPTS, OUTW], f32, kind="ExternalOutput")
    stats_in_d = nc.dram_tensor("stats_in", [128, 64], f32)
    stats_out_d = nc.dram_tensor("stats_out", [128, 64], f32, addr_space="Shared")

    with ExitStack() as ctx:
        tc = ctx.enter_context(tile.TileContext(nc))
        big = ctx.enter_context(tc.tile_pool(name="big", bufs=1))
        consts = ctx.enter_context(tc.tile_pool(name="consts", bufs=1))
        stat = ctx.enter_context(tc.tile_pool(name="stat", bufs=1))
        work = ctx.enter_context(tc.tile_pool(name="work", bufs=3))
        x2p = ctx.enter_context(tc.tile_pool(name="x2p", bufs=4))
        outp = ctx.enter_context(tc.tile_pool(name="outp", bufs=3))
        dvw = ctx.enter_context(tc.tile_pool(name="dvw", bufs=3))
        psA = ctx.enter_context(tc.tile_pool(name="psA", bufs=2, space="PSUM"))
        psB = ctx.enter_context(tc.tile_pool(name="psB", bufs=1, space="PSUM"))
        psC = ctx.enter_context(tc.tile_pool(name="psC", bufs=1, space="PSUM"))
        psS = ctx.enter_context(tc.tile_pool(name="psS", bufs=1, space="PSUM"))

        # ---------------- loads ----------------
        fnat = big.tile([128, MCH, KF], bf16)
        nc.sync.dma_start(out=fnat, in_=fnat_d[:])
        featT = big.tile([128, 2, NPTS], bf16)
        nc.sync.dma_start(out=featT, in_=featT_d[:])
        w1 = big.tile([128, 2, NCH], bf16)
        nc.sync.dma_start(out=w1, in_=w1_d[:])
        wbig = big.tile([128, MCH, U], bf16)
        nc.sync.dma_start(out=wbig, in_=wbig_d[:])
        gam = consts.tile([128, MCH], f32)
        nc.sync.dma_start(out=gam, in_=gam_d[:])
        bet = consts.tile([128, MCH], f32)
        nc.sync.dma_start(out=bet, in_=bet_d[:])
        catf = consts.tile([128, NPTS // 128], f32)
        nc.sync.dma_start(out=catf, in_=catf_d[:])
        catid96 = consts.tile([128, U], f32)
        nc.sync.dma_start(out=catid96, in_=catid_d[:].to_broadcast((128, U)))
        bias96 = consts.tile([128, U], f32)
        nc.sync.dma_start(out=bias96, in_=bias_d[:].to_broadcast((128, U)))
        ones = consts.tile([128, 1], bf16)
        nc.vector.memset(ones, 1.0)
        ident = consts.tile([128, 128], f32)
        make_identity(nc, ident)


        # ---------------- phase 0: Gram matrix + column sums ----------------
        # C[l, k] = sum_n F[n, l] F[n, k]   (two 128-row halves of l)
        C_sb = stat.tile([128, 2, KF], bf16)  # [l-part, l-half, k]
        for kc in range(2):
            pc = psS.tile([128, KF], f32, tag="ph0")
            for i in range(MCH):
                nc.tensor.matmul(
                    pc,
                    lhsT=(fnat[:, i, kc * 128:(kc + 1) * 128]),
                    rhs=(fnat[:, i, :]),
                    start=(i == 0),
                    stop=(i == MCH - 1),
                )
            nc.vector.tensor_copy(out=C_sb[:, kc, :], in_=pc)

        # sT[k] = sum_n F[n, k], laid out [128, 2] (col = k-half)
        sT = stat.tile([128, 2], bf16)
        for kc in range(2):
            ps = psS.tile([128, 1], f32, tag="ph0")
            for i in range(MCH):
                nc.tensor.matmul(
                    ps,
                    lhsT=(fnat[:, i, kc * 128:(kc + 1) * 128]),
                    rhs=(ones),
                    start=(i == 0),
                    stop=(i == MCH - 1),
                )
            nc.vector.tensor_copy(out=sT[:, kc:kc + 1], in_=ps)

        # D = C @ W1 ; P = W1 * D (elementwise); held as 2 k-chunks [128, NCH]
        P_sb = stat.tile([128, 2, NCH], bf16)
        for kc in range(2):  # k-chunk (row half of D)
            for nb in range(NCH // 512):
                pd = psA.tile([128, 512], f32, tag="x1")
                for ll in range(2):  # contraction over l
                    nc.tensor.matmul(
                        pd,
                        lhsT=(C_sb[:, ll, kc * 128:(kc + 1) * 128]),
                        rhs=(w1[:, ll, nb * 512:(nb + 1) * 512]),
                        start=(ll == 0),
                        stop=(ll == 1),
                    )
                nc.vector.tensor_mul(
                    out=P_sb[:, kc, nb * 512:(nb + 1) * 512],
                    in0=pd,
                    in1=w1[:, kc, nb * 512:(nb + 1) * 512],
                )

        # per-channel sums in [128, 32] layout (partition p, chunk m -> ch m*128+p)
        # sumsq[j] = sum_k P[k, j] ; sumx1[j] = sum_k s[k] W1[k, j]
        stats_sb = stat.tile([128, 64], f32)
        p_sq = psS.tile([128, MCH], f32, tag="ph0")
        for jb in range(MCH):
            for kc in range(2):
                nc.tensor.matmul(
                    p_sq[:, jb:jb + 1],
                    lhsT=(P_sb[:, kc, jb * 128:(jb + 1) * 128]),
                    rhs=(ones),
                    start=(kc == 0),
                    stop=(kc == 1),
                )
        nc.vector.tensor_copy(out=stats_sb[:, MCH:2 * MCH], in_=p_sq)
        p_sx = psS.tile([128, MCH], f32, tag="ph0")
        for jb in range(MCH):
            for kc in range(2):
                nc.tensor.matmul(
                    p_sx[:, jb:jb + 1],
                    lhsT=(w1[:, kc, jb * 128:(jb + 1) * 128]),
                    rhs=(sT[:, kc:kc + 1]),
                    start=(kc == 0),
                    stop=(kc == 1),
                )
        nc.vector.tensor_copy(out=stats_sb[:, 0:MCH], in_=p_sx)

        wr = nc.sync.dma_start(out=stats_in_d[:], in_=stats_sb)
        cc = nc.gpsimd.collective_compute(
            "AllReduce",
            ALU.add,
            replica_groups=[list(range(NCORES))],
            ins=[stats_in_d[:]],
            outs=[stats_out_d[:]],
        )
        add_dep_helper(cc.ins, wr.ins, reason="stats written before allreduce")

        # Overlap the AllReduce latency: precompute pb0's raw x1 into SBUF
        # (normalized later once BN stats arrive).
        x1raw = big.tile([128, MCH, NB], bf16)
        for m in range(MCH):
            px = psA.tile([128, NB], f32, tag="x1")
            for h in range(NHALF):
                for kc in range(2):
                    nc.tensor.matmul(
                        px[:, h * 512:(h + 1) * 512],
                        lhsT=(w1[:, kc, m * 128:(m + 1) * 128]),
                        rhs=(featT[:, kc, h * 512:(h + 1) * 512]),
                        start=(kc == 0),
                        stop=(kc == 1),
                    )
            nc.vector.tensor_copy(out=x1raw[:, m, :], in_=px)

        stats_g = stat.tile([128, 64], f32)
        rd = nc.sync.dma_start(out=stats_g, in_=stats_out_d[:])
        add_dep_helper(rd.ins, cc.ins, reason="allreduce before readback")

        # ---------------- a, b (per-channel affine of BN + gamma/beta) -------
        mu = stat.tile([128, MCH], f32)
        nc.vector.tensor_scalar(
            out=mu, in0=stats_g[:, 0:MCH], scalar1=1.0 / N_GLOBAL, scalar2=None,
            op0=ALU.mult,
        )
        var = stat.tile([128, MCH], f32)
        # var = sumsq/N - mu^2
        nc.vector.tensor_scalar(
            out=var, in0=stats_g[:, MCH:2 * MCH], scalar1=1.0 / N_GLOBAL,
            scalar2=None, op0=ALU.mult,
        )
        mu2 = stat.tile([128, MCH], f32)
        nc.vector.tensor_mul(out=mu2, in0=mu, in1=mu)
        nc.vector.tensor_sub(out=var, in0=var, in1=mu2)
        eps_t = stat.tile([128, 1], f32)
        nc.vector.memset(eps_t, BN_EPS)
        std = stat.tile([128, MCH], f32)
        nc.scalar.activation(out=std, in_=var, func=AF.Sqrt, bias=eps_t, scale=1.0)
        rstd = stat.tile([128, MCH], f32)
        nc.vector.reciprocal(out=rstd, in_=std)
        a_t = stat.tile([128, MCH], f32)
        nc.vector.tensor_mul(out=a_t, in0=gam, in1=rstd)
        b_t = stat.tile([128, MCH], f32)
        nc.vector.tensor_mul(out=b_t, in0=mu, in1=a_t)
        nc.vector.tensor_sub(out=b_t, in0=bet, in1=b_t)

        # ---------------- main loop ----------------
        for pb in range(NPB):
            pf = psB.tile([U, NB], f32, tag="feats")
            for m in range(MCH):
                if pb == 0:
                    px = x1raw[:, m, :]
                else:
                    px = psA.tile([128, NB], f32, tag="x1")
                    for h in range(NHALF):
                        for kc in range(2):
                            nc.tensor.matmul(
                                px[:, h * 512:(h + 1) * 512],
                                lhsT=(w1[:, kc, m * 128:(m + 1) * 128]),
                                rhs=(featT[:, kc,
                                           pb * NB + h * 512:pb * NB + (h + 1) * 512]),
                                start=(kc == 0),
                                stop=(kc == 1),
                            )
                x2 = x2p.tile([128, NB], bf16, tag="x2")
                _normalize(nc, dvw, x2, px, a_t, b_t, m)
                for h in range(NHALF):
                    nc.tensor.matmul(
                        pf[:, h * 512:(h + 1) * 512],
                        lhsT=(wbig[:, m, :]),
                        rhs=(x2[:, h * 512:(h + 1) * 512]),
                        start=(m == 0),
                        stop=(m == MCH - 1),
                    )
            feats_sb = work.tile([U, NB], f32, tag="feats_sb")
            nc.vector.tensor_copy(out=feats_sb, in_=pf)

            obuf = outp.tile([128, NB // 128, OUTW], f32, tag="obuf")
            for i in range(NB // 128):
                sbk = pb * (NB // 128) + i
                pT = psC.tile([128, U], f32, tag="pT")
                nc.tensor.transpose(
                    pT, feats_sb[:, i * 128:(i + 1) * 128], ident[0:U, 0:U]
                )
                mask = outp.tile([128, U], f32, tag="mask")
                nc.vector.tensor_tensor(
                    out=mask, in0=catid96,
                    in1=catf[:, sbk:sbk + 1].to_broadcast((128, U)),
                    op=ALU.is_equal,
                )
                t2 = outp.tile([128, U], f32, tag="t2")
                nc.vector.tensor_add(out=t2, in0=pT, in1=bias96)
                nc.vector.tensor_mul(out=t2, in0=t2, in1=mask)
                nmx = outp.tile([128, 1], f32, tag="nmx")
                nc.vector.tensor_reduce(
                    out=nmx, in_=t2, axis=mybir.AxisListType.X, op=ALU.max,
                    negate=True,
                )
                e = outp.tile([128, U], f32, tag="e")
                se = outp.tile([128, 1], f32, tag="se")
                nc.scalar.activation(
                    out=e, in_=t2, func=AF.Exp, bias=nmx, scale=1.0, accum_out=se
                )
                lse = outp.tile([128, 1], f32, tag="lse")
                nc.scalar.activation(out=lse, in_=se, func=AF.Ln)
                h = outp.tile([128, 1], f32, tag="h")
                nc.vector.tensor_sub(out=h, in0=lse, in1=nmx)  # lse + max
                nc.vector.tensor_tensor(
                    out=obuf[:, i, :], in0=t2[:, 0:OUTW],
                    in1=h.to_broadcast((128, OUTW)),
                    op=ALU.subtract,
                )
                nc.vector.tensor_mul(
                    out=obuf[:, i, :], in0=obuf[:, i, :], in1=mask[:, 0:OUTW]
                )
            dst = out_d[pb * NB:(pb + 1) * NB, :].rearrange(
                "(i p) w -> p i w", p=128
            )
            nc.sync.dma_start(out=dst, in_=obuf)

    if not nc.is_finalized():
        nc.finalize()
    return nc


@functools.lru_cache(maxsize=1)
def _get_program():
    return build_program()


def _host_prep(features, W1, gamma, beta, Wc, bias, cats, shifts, seg_lens):
    features = np.ascontiguousarray(np.asarray(features, dtype=np.float32))
    W1 = np.ascontiguousarray(np.asarray(W1, dtype=np.float32))
    gamma = np.asarray(gamma, dtype=np.float32)
    beta = np.asarray(beta, dtype=np.float32)
    Wc = np.asarray(Wc, dtype=np.float32)
    bias = np.asarray(bias, dtype=np.float32)
    cats = np.asarray(cats)
    shifts = np.asarray(shifts).astype(np.int64)
    seg_lens = np.asarray(seg_lens).astype(np.int64)

    # expanded column layout: 0..49 are the output columns, 50..95 overflow
    cat_of_u = np.zeros(U, np.int64)
    s_of_u = np.zeros(U, np.int64)
    for c in range(NCAT):
        for j in range(int(seg_lens[c])):
            k = int(shifts[c]) + j
            cat_of_u[k] = c
            s_of_u[k] = j
    idx = OUTW
    for c in range(NCAT):
        for j in range(int(seg_lens[c]), SEG):
            cat_of_u[idx] = c
            s_of_u[idx] = j
            idx += 1
    assert idx == U

    Wbig = np.zeros((NCAT, KF, U), np.float32)
    for u in range(U):
        Wbig[cat_of_u[u], :, u] = Wc[cat_of_u[u], :, s_of_u[u]]
    Wbig = Wbig.reshape(NCH, U)

    common = {
        "w1": np.ascontiguousarray(
            W1.reshape(2, 128, NCH).transpose(1, 0, 2)
        ).astype(BF),
        "wbig": np.ascontiguousarray(
            Wbig.reshape(MCH, 128, U).transpose(1, 0, 2)
        ).astype(BF),
        "gamma_t": np.ascontiguousarray(gamma.reshape(MCH, 128).T),
        "beta_t": np.ascontiguousarray(beta.reshape(MCH, 128).T),
        "catid96": cat_of_u.astype(np.float32).reshape(1, U),
        "bias96": (bias[s_of_u] + BIG).astype(np.float32).reshape(1, U),
    }

    in_maps = []
    for ci in range(NCORES):
        fc = features[ci * NPTS:(ci + 1) * NPTS]
        catc = cats[ci * NPTS:(ci + 1) * NPTS]
        m = dict(common)
        m["featT"] = np.ascontiguousarray(
            fc.T.reshape(2, 128, NPTS).transpose(1, 0, 2)
        ).astype(BF)
        m["fnat"] = np.ascontiguousarray(
            fc.reshape(MCH, 128, KF).transpose(1, 0, 2)
        ).astype(BF)
        m["catf"] = np.ascontiguousarray(
            catc.reshape(NPTS // 128, 128).T.astype(np.float32)
        )
        in_maps.append(m)
    return in_maps


def kernel(**inputs):
    in_maps = _host_prep(
        inputs["features"], inputs["W1"], inputs["gamma"], inputs["beta"],
        inputs["Wc"], inputs["bias"], inputs["cats"], inputs["shifts"],
        inputs["seg_lens"],
    )
    nc = _get_program()
    res = run_bass_kernel_spmd(nc, in_maps, core_ids=list(range(NCORES)))
    out = np.concatenate(
        [res.results[i]["out"] for i in range(NCORES)], axis=0
    )
    return np.ascontiguousarray(out.astype(np.float32))


# used by test.py for profiling runs
def kernel_traced(**inputs):
    in_maps = _host_prep(
        inputs["features"], inputs["W1"], inputs["gamma"], inputs["beta"],
        inputs["Wc"], inputs["bias"], inputs["cats"], inputs["shifts"],
        inputs["seg_lens"],
    )
    nc = _get_program()
    res = run_bass_kernel_spmd(
        nc, in_maps, core_ids=list(range(NCORES)), trace=True
    )
    out = np.concatenate(
        [res.results[i]["out"] for i in range(NCORES)], axis=0
    )
    return np.ascontiguousarray(out.astype(np.float32)), res



# revision 25
# speedup vs baseline: 2.3137x; 2.3137x over previous
"""Trainium2 Bass kernel for nn_MultiHeadClassifier (moe_routing).

Strategy: TRUE ROUTING + analytic BatchNorm statistics.

The reference's dense x1 = features @ W1 (all C*F=4096 channels for all
N points) is needed for two things only:
  (1) BatchNorm batch statistics over ALL points for EVERY channel, and
  (2) each point's OWN category's 256 channels (which feed the head).

(2) is 1/16th of the dense work: host prep sorts points by category
(pure data movement, like any sharding) into 16 x CAP padded slots per
core and the device computes x1 only for routed (point, own-category-
channel) pairs.

(1) cannot use routed sums (channel j's stats include points of OTHER
categories), so it is computed analytically from the feature Gram
matrix, linear in per-core partials -> one 32KB AllReduce:

    sumx1_j = s^T W1[:, j],            s = sum_n F[n, :]
    sumsq_j = sum_k W1[k, j] * D^T[j, k],  D^T = W1^T C, C = F^T F

D^T (channels on partitions) lets the DVE's fused tensor_tensor_reduce
produce both stats directly in the [128, 32] per-partition layout the
normalize step wants -- no transposes or rearranging DMAs.

The head uses a block-diagonal trick: one [128, 96] stationary weight
computes all 16 categories' 6 logits in wide 512-column matmuls (row
group 6c is garbage for columns of other categories -- never read).
Grouped log-softmax runs entirely in wide ops via 96->16 selector
matmuls. Output is [96, P_ALL]; the host scatters the valid (category,
segment) entries into the [N, 50] result.
"""

import os
import sys
import functools
from contextlib import ExitStack

import numpy as np
import ml_dtypes

BF = ml_dtypes.bfloat16

for _p in ("/opt/trn_rl_repo", "/root/.axon_site/_ro/trn_rl_repo"):
    if os.path.isdir(_p) and _p not in sys.path:
        sys.path.insert(0, _p)

import concourse.bass as bass
import concourse.tile as tile
from concourse import bacc
from concourse import mybir
from concourse.bass_utils import run_bass_kernel_spmd
from concourse.masks import make_identity
from concourse.tile_rust import add_dep_helper

NCORES = 8
NPTS = 4096          # points per core (contiguous shard, for Gram stats)
NCH = 4096           # C*F hidden channels
KF = 256             # input features
NCAT = 16
SEG = 6
CAP = 320            # routed slots per (core, category)
P_ALL = NCAT * CAP   # 5120 padded routed points per core
U = NCAT * SEG       # 96 stacked logit rows
MCH = NCH // 128     # 32 channel chunks
N_GLOBAL = NCORES * NPTS
BN_EPS = 1e-5
LEAK = 0.2
CHK = 512            # head/softmax column chunk
NCHK = P_ALL // CHK  # 10
OUTW = 50

f32 = mybir.dt.float32
bf16 = mybir.dt.bfloat16
AF = mybir.ActivationFunctionType
ALU = mybir.AluOpType


# CoreSim does not implement the Prelu activation; flip this off (and clear
# _get_program's cache) to build a sim-compatible program for debugging.
USE_PRELU = True


class _Bacc(bacc.Bacc):
    """Keep only natural_log_exp_and_others (exp, ln, parametric_relu,
    copy) so the ACT engine loads exactly one table, once."""

    def insert_act_table_loads(self):
        import bass_rust as _br
        from concourse.hw_specs import get_activation_tables
        has_activation = any(
            isinstance(i, mybir.InstActivation)
            for b in self.main_func.blocks
            for i in b.instructions
        )
        if not has_activation:
            return
        keep = ("natural_log_exp_and_others",)
        tables = [
            (name, funcs if name in keep else set())
            for name, funcs in get_activation_tables(self.m.arch).items()
        ]
        _br.insert_act_table_loads(self, tables)


def build_program():
    nc = _Bacc()

    fnat_d = nc.dram_tensor("fnat", [128, MCH, KF], bf16, kind="ExternalInput")
    fgT_d = nc.dram_tensor("fgT", [128, 2, P_ALL], bf16, kind="ExternalInput")
    w1_d = nc.dram_tensor("w1", [128, 2, NCH], bf16, kind="ExternalInput")
    wnat_d = nc.dram_tensor("wnat", [128, MCH, KF], bf16, kind="ExternalInput")
    wcb_d = nc.dram_tensor("wcb", [128, 2, U], bf16, kind="ExternalInput")
    gam_d = nc.dram_tensor("gamma_t", [128, MCH], f32, kind="ExternalInput")
    bet_d = nc.dram_tensor("beta_t", [128, MCH], f32, kind="ExternalInput")
    r96_d = nc.dram_tensor("r96", [U, U], bf16, kind="ExternalInput")
    bias_d = nc.dram_tensor("bias96", [U, 1], f32, kind="ExternalInput")
    out_d = nc.dram_tensor("out", [U, P_ALL], f32, kind="ExternalOutput")
    stats_in_d = nc.dram_tensor("stats_in", [128, 64], f32)
    stats_out_d = nc.dram_tensor("stats_out", [128, 64], f32, addr_space="Shared")

    with ExitStack() as ctx:
        tc = ctx.enter_context(tile.TileContext(nc))
        big = ctx.enter_context(tc.tile_pool(name="big", bufs=1))
        consts = ctx.enter_context(tc.tile_pool(name="consts", bufs=1))
        stat = ctx.enter_context(tc.tile_pool(name="stat", bufs=1))
        scrp = ctx.enter_context(tc.tile_pool(name="scrp", bufs=2))
        work = ctx.enter_context(tc.tile_pool(name="work", bufs=3))
        psC = ctx.enter_context(tc.tile_pool(name="psC", bufs=1, space="PSUM"))
        psA = ctx.enter_context(tc.tile_pool(name="psA", bufs=2, space="PSUM"))
        psH = ctx.enter_context(tc.tile_pool(name="psH", bufs=2, space="PSUM"))
        psS = ctx.enter_context(tc.tile_pool(name="psS", bufs=1, space="PSUM"))
        psX = ctx.enter_context(tc.tile_pool(name="psX", bufs=1, space="PSUM"))

        # ---------------- loads --------------------------------------------
        wcb = consts.tile([128, 2, U], bf16)
        nc.sync.dma_start(out=wcb, in_=wcb_d[:])
        gam = consts.tile([128, MCH], f32)
        nc.sync.dma_start(out=gam, in_=gam_d[:])
        bet = consts.tile([128, MCH], f32)
        nc.sync.dma_start(out=bet, in_=bet_d[:])
        r96 = consts.tile([U, U], bf16)
        nc.sync.dma_start(out=r96, in_=r96_d[:])
        bias96 = consts.tile([U, 1], f32)
        nc.sync.dma_start(out=bias96, in_=bias_d[:])

        fnat = big.tile([128, MCH, KF], bf16)
        for q in range(4):
            nc.sync.dma_start(
                out=fnat[:, q * 8:(q + 1) * 8, :],
                in_=fnat_d[:, q * 8:(q + 1) * 8, :],
            )
        fgT = big.tile([128, 2, P_ALL], bf16)
        nc.sync.dma_start(out=fgT, in_=fgT_d[:])
        w1 = big.tile([128, 2, NCH], bf16)
        nc.sync.dma_start(out=w1, in_=w1_d[:])
        wnat = big.tile([128, MCH, KF], bf16)
        nc.sync.dma_start(out=wnat, in_=wnat_d[:])

        eps_t = consts.tile([128, 1], f32)
        nc.vector.memset(eps_t, BN_EPS)

        # ------------- s = sum_n F[n, :] over this core's routed points ----
        s_f = stat.tile([128, 2], f32)
        nc.vector.tensor_reduce(
            out=s_f, in_=fgT, axis=mybir.AxisListType.X, op=ALU.add,
        )
        s_bf = stat.tile([128, 2], bf16)
        nc.vector.tensor_copy(out=s_bf, in_=s_f)

        # ------------- C = F^T F (interleaved kc so it starts on the first
        # fnat DMA chunk) ---------------------------------------------------
        pc0 = psC.tile([128, KF], f32, tag="pc0")
        pc1 = psC.tile([128, KF], f32, tag="pc1")
        for i in range(MCH):
            for kc in range(2):
                nc.tensor.matmul(
                    pc0 if kc == 0 else pc1,
                    lhsT=fnat[:, i, kc * 128:(kc + 1) * 128],
                    rhs=fnat[:, i, :],
                    start=(i == 0),
                    stop=(i == MCH - 1),
                )
        C_sb = stat.tile([128, 2, KF], bf16)
        nc.vector.tensor_copy(out=C_sb[:, 0, :], in_=pc0)
        nc.vector.tensor_copy(out=C_sb[:, 1, :], in_=pc1)

        # stats tile: [0:32] sumx1 partial, [32:64] sumsq partial
        stats_sb = stat.tile([128, 64], f32)
        # sumx1 partial: stats column m = W1 chunk^T s (baseline p_sx shape)
        p_sx = psX.tile([128, MCH], f32, tag="psx")
        for m in range(MCH):
            for kc in range(2):
                nc.tensor.matmul(
                    p_sx[:, m:m + 1],
                    lhsT=w1[:, kc, m * 128:(m + 1) * 128],
                    rhs=s_bf[:, kc:kc + 1],
                    start=(kc == 0),
                    stop=(kc == 1),
                )
        nc.vector.tensor_copy(out=stats_sb[:, 0:MCH], in_=p_sx)

        # ------------- D^T = W1^T C with channels on partitions; multiply-
        # reduce against W1nat gives sumsq directly in stats layout ---------
        for m in range(MCH):
            pdt = psA.tile([128, CAP], f32, tag="x1")
            pd = pdt[:, 0:KF]
            for ll in range(2):
                nc.tensor.matmul(
                    pd,
                    lhsT=w1[:, ll, m * 128:(m + 1) * 128],
                    rhs=C_sb[:, ll, :],
                    start=(ll == 0),
                    stop=(ll == 1),
                )
            scr = scrp.tile([128, KF], bf16, tag="sq")
            nc.vector.tensor_mul(out=scr, in0=pd, in1=wnat[:, m, :])
            nc.vector.tensor_reduce(
                out=stats_sb[:, 32 + m:32 + m + 1], in_=scr,
                axis=mybir.AxisListType.X, op=ALU.add,
            )

        wr = nc.sync.dma_start(out=stats_in_d[:], in_=stats_sb)
        cc = nc.gpsimd.collective_compute(
            "AllReduce",
            ALU.add,
            replica_groups=[list(range(NCORES))],
            ins=[stats_in_d[:]],
            outs=[stats_out_d[:]],
        )
        add_dep_helper(cc.ins, wr.ins, reason="stats written before allreduce")

        # ------------- routed x1 (runs under the AllReduce) -----------------
        # chunk (c, kc) = channels c*256 + kc*128 + p
        x1raw = big.tile([128, 2, P_ALL], bf16)   # [p, kc, c*CAP+s]
        for c in range(NCAT):
            for kc in range(2):
                px = psA.tile([128, CAP], f32, tag="x1")
                for ki in range(2):
                    nc.tensor.matmul(
                        px,
                        lhsT=w1[:, ki, c * 256 + kc * 128:c * 256 + kc * 128 + 128],
                        rhs=fgT[:, ki, c * CAP:(c + 1) * CAP],
                        start=(ki == 0),
                        stop=(ki == 1),
                    )
                if (2 * c + kc) % 2 == 0:
                    nc.vector.tensor_copy(
                        out=x1raw[:, kc, c * CAP:(c + 1) * CAP], in_=px
                    )
                else:
                    nc.scalar.copy(
                        out=x1raw[:, kc, c * CAP:(c + 1) * CAP], in_=px
                    )

        stats_g = stat.tile([128, 64], f32)
        rd = nc.sync.dma_start(out=stats_g, in_=stats_out_d[:])
        add_dep_helper(rd.ins, cc.ins, reason="allreduce before readback")

        # ---------------- a, b (BN affine) ----------------------------------
        mu = stat.tile([128, MCH], f32)
        nc.vector.tensor_scalar(
            out=mu, in0=stats_g[:, 0:MCH], scalar1=1.0 / N_GLOBAL, scalar2=None,
            op0=ALU.mult,
        )
        var = stat.tile([128, MCH], f32)
        nc.vector.tensor_scalar(
            out=var, in0=stats_g[:, MCH:2 * MCH], scalar1=1.0 / N_GLOBAL,
            scalar2=None, op0=ALU.mult,
        )
        mu2 = stat.tile([128, MCH], f32)
        nc.vector.tensor_mul(out=mu2, in0=mu, in1=mu)
        nc.vector.tensor_sub(out=var, in0=var, in1=mu2)
        # rstd = exp(-0.5 * ln(var + eps)) -- stays on the single ACT table
        lnv = stat.tile([128, MCH], f32)
        nc.scalar.activation(out=lnv, in_=var, func=AF.Ln, bias=eps_t, scale=1.0)
        rstd = stat.tile([128, MCH], f32)
        nc.scalar.activation(out=rstd, in_=lnv, func=AF.Exp, scale=-0.5)
        a_t = stat.tile([128, MCH], f32)
        nc.vector.tensor_mul(out=a_t, in0=gam, in1=rstd)
        b_t = stat.tile([128, MCH], f32)
        nc.vector.tensor_mul(out=b_t, in0=mu, in1=a_t)
        nc.vector.tensor_sub(out=b_t, in0=bet, in1=b_t)

        # ---------------- normalize: x2 = LeakyReLU(a*x1 + b) ----------------
        x2all = big.tile([128, 2, P_ALL], bf16)
        for c in range(NCAT):
            for kc in range(2):
                m = 2 * c + kc
                src = x1raw[:, kc, c * CAP:(c + 1) * CAP]
                dst = x2all[:, kc, c * CAP:(c + 1) * CAP]
                if USE_PRELU and m % 16 < 10:
                    nc.scalar.activation(
                        out=dst, in_=src, func=AF.Prelu,
                        bias=b_t[:, m:m + 1], scale=a_t[:, m:m + 1], alpha=LEAK,
                    )
                else:
                    y = scrp.tile([128, CAP], bf16, tag="y")
                    nc.vector.tensor_scalar(
                        out=y, in0=src, scalar1=a_t[:, m:m + 1],
                        scalar2=b_t[:, m:m + 1], op0=ALU.mult, op1=ALU.add,
                    )
                    t02 = scrp.tile([128, CAP], bf16, tag="t02")
                    nc.vector.tensor_scalar_mul(out=t02, in0=y, scalar1=LEAK)
                    nc.vector.tensor_tensor(out=dst, in0=y, in1=t02, op=ALU.max)

        # ------------- head + grouped log-softmax, 512-col chunks ------------
        for t in range(NCHK):
            lo = t * CHK
            pf = psH.tile([U, CHK], f32, tag="pf")
            for kc in range(2):
                nc.tensor.matmul(
                    pf,
                    lhsT=wcb[:, kc, :],
                    rhs=x2all[:, kc, lo:lo + CHK],
                    start=(kc == 0),
                    stop=(kc == 1),
                )
            e = work.tile([U, CHK], bf16, tag="e")
            nc.scalar.activation(
                out=e, in_=pf, func=AF.Exp, bias=bias96, scale=1.0
            )
            # R96 (block-diag ones) sums each 6-row group AND broadcasts the
            # sum back to all 6 rows in one matmul
            ps = psS.tile([U, CHK], f32, tag="ps")
            nc.tensor.matmul(ps, lhsT=r96, rhs=e, start=True, stop=True)
            lnS = work.tile([U, CHK], bf16, tag="lnS")
            nc.scalar.activation(out=lnS, in_=ps, func=AF.Ln)
            tb = work.tile([U, CHK], f32, tag="tb")
            nc.vector.tensor_scalar(
                out=tb, in0=pf, scalar1=bias96, scalar2=None, op0=ALU.add,
            )
            lsm = work.tile([U, CHK], f32, tag="lsm")
            nc.vector.tensor_tensor(out=lsm, in0=tb, in1=lnS, op=ALU.subtract)
            nc.sync.dma_start(out=out_d[:, lo:lo + CHK], in_=lsm)

    if not nc.is_finalized():
        nc.finalize()
    return nc


@functools.lru_cache(maxsize=1)
def _get_program():
    return build_program()


def _host_prep(features, W1, gamma, beta, Wc, bias, cats, shifts, seg_lens):
    features = np.ascontiguousarray(np.asarray(features, dtype=np.float32))
    W1 = np.ascontiguousarray(np.asarray(W1, dtype=np.float32))
    gamma = np.asarray(gamma, dtype=np.float32)
    beta = np.asarray(beta, dtype=np.float32)
    Wc = np.asarray(Wc, dtype=np.float32)
    bias = np.asarray(bias, dtype=np.float32)
    cats = np.asarray(cats)

    # route: global sort by category, split each category across the 8 cores
    order = np.argsort(cats, kind="stable")
    counts = np.bincount(cats, minlength=NCAT)
    starts = np.concatenate([[0], np.cumsum(counts)[:-1]])
    gidx = [[None] * NCAT for _ in range(NCORES)]
    for c in range(NCAT):
        pts = order[starts[c]:starts[c] + counts[c]]
        splits = np.array_split(pts, NCORES)
        for ci in range(NCORES):
            assert len(splits[ci]) <= CAP, (
                f"category {c} count {counts[c]} exceeds capacity"
            )
            gidx[ci][c] = splits[ci]

    # wcb[p, kc, 6c+j] = Wc[c, kc*128+p, j]
    wcb = np.zeros((128, 2, U), np.float32)
    for c in range(NCAT):
        for kc in range(2):
            wcb[:, kc, 6 * c:6 * c + 6] = Wc[c, kc * 128:(kc + 1) * 128, :]

    r96 = np.zeros((U, U), np.float32)
    for c in range(NCAT):
        r96[6 * c:6 * c + 6, 6 * c:6 * c + 6] = 1.0

    common = {
        "w1": np.ascontiguousarray(
            W1.reshape(2, 128, NCH).transpose(1, 0, 2)
        ).astype(BF),
        "wnat": np.ascontiguousarray(
            W1.T.reshape(MCH, 128, KF).transpose(1, 0, 2)
        ).astype(BF),
        "wcb": wcb.astype(BF),
        "gamma_t": np.ascontiguousarray(gamma.reshape(MCH, 128).T),
        "beta_t": np.ascontiguousarray(beta.reshape(MCH, 128).T),
        "r96": r96.astype(BF),
        "bias96": np.tile(bias, NCAT).astype(np.float32).reshape(U, 1),
    }

    fT = features.T.astype(BF)  # [256, N]
    in_maps = []
    for ci in range(NCORES):
        fc = features[ci * NPTS:(ci + 1) * NPTS]
        fg = np.zeros((128, 2, P_ALL), BF)
        for c in range(NCAT):
            g = gidx[ci][c]
            blk = fT[:, g].reshape(2, 128, len(g))
            fg[:, :, c * CAP:c * CAP + len(g)] = blk.transpose(1, 0, 2)
        m = dict(common)
        m["fgT"] = fg
        m["fnat"] = np.ascontiguousarray(
            fc.reshape(MCH, 128, KF).transpose(1, 0, 2)
        ).astype(BF)
        in_maps.append(m)
    return in_maps, gidx


def _assemble(results, gidx, shifts, seg_lens):
    shifts = np.asarray(shifts).astype(np.int64)
    seg_lens = np.asarray(seg_lens).astype(np.int64)
    out = np.zeros((NCORES * NPTS, OUTW), np.float32)
    for ci in range(NCORES):
        lsm = results[ci]["out"]  # [U, P_ALL] f32
        for c in range(NCAT):
            g = gidx[ci][c]
            if len(g) == 0:
                continue
            ln = int(seg_lens[c])
            sh = int(shifts[c])
            blk = lsm[6 * c:6 * c + ln, c * CAP:c * CAP + len(g)]
            out[np.ix_(g, np.arange(sh, sh + ln))] = blk.T
    return out


def kernel(**inputs):
    in_maps, gidx = _host_prep(
        inputs["features"], inputs["W1"], inputs["gamma"], inputs["beta"],
        inputs["Wc"], inputs["bias"], inputs["cats"], inputs["shifts"],
        inputs["seg_lens"],
    )
    nc = _get_program()
    res = run_bass_kernel_spmd(nc, in_maps, core_ids=list(range(NCORES)))
    return _assemble(res.results, gidx, inputs["shifts"], inputs["seg_lens"])


# used by test.py for profiling runs
def kernel_traced(**inputs):
    in_maps, gidx = _host_prep(
        inputs["features"], inputs["W1"], inputs["gamma"], inputs["beta"],
        inputs["Wc"], inputs["bias"], inputs["cats"], inputs["shifts"],
        inputs["seg_lens"],
    )
    nc = _get_program()
    res = run_bass_kernel_spmd(
        nc, in_maps, core_ids=list(range(NCORES)), trace=True
    )
    out = _assemble(res.results, gidx, inputs["shifts"], inputs["seg_lens"])
    return out, res


# revision 31
# speedup vs baseline: 2.5301x; 1.0935x over previous
"""Trainium2 Bass kernel for nn_MultiHeadClassifier (moe_routing).

Strategy: TRUE ROUTING + analytic BatchNorm statistics.

The reference's dense x1 = features @ W1 (all C*F=4096 channels for all
N points) is needed for two things only:
  (1) BatchNorm batch statistics over ALL points for EVERY channel, and
  (2) each point's OWN category's 256 channels (which feed the head).

(2) is 1/16th of the dense work: host prep sorts points by category
(pure data movement, like any sharding) into 16 x CAP padded slots per
core and the device computes x1 only for routed (point, own-category-
channel) pairs.

(1) cannot use routed sums (channel j's stats include points of OTHER
categories), so it is computed analytically from the feature Gram
matrix, linear in per-core partials -> one 32KB AllReduce:

    sumx1_j = s^T W1[:, j],            s = sum_n F[n, :]
    sumsq_j = sum_k W1[k, j] * D^T[j, k],  D^T = W1^T C, C = F^T F

D^T (channels on partitions) lets the DVE's fused tensor_tensor_reduce
produce both stats directly in the [128, 32] per-partition layout the
normalize step wants -- no transposes or rearranging DMAs.

The head uses a block-diagonal trick: one [128, 96] stationary weight
computes all 16 categories' 6 logits in wide 512-column matmuls (row
group 6c is garbage for columns of other categories -- never read).
Grouped log-softmax runs entirely in wide ops via 96->16 selector
matmuls. Output is [96, P_ALL]; the host scatters the valid (category,
segment) entries into the [N, 50] result.
"""

import os
import sys
import functools
from contextlib import ExitStack

import numpy as np
import ml_dtypes

BF = ml_dtypes.bfloat16

for _p in ("/opt/trn_rl_repo", "/root/.axon_site/_ro/trn_rl_repo"):
    if os.path.isdir(_p) and _p not in sys.path:
        sys.path.insert(0, _p)

import concourse.bass as bass
import concourse.tile as tile
from concourse import bacc
from concourse import mybir
from concourse.bass_utils import run_bass_kernel_spmd
from concourse.masks import make_identity
from concourse.tile_rust import add_dep_helper

NCORES = 8
NPTS = 4096          # points per core (contiguous shard, for Gram stats)
NCH = 4096           # C*F hidden channels
KF = 256             # input features
NCAT = 16
SEG = 6
CAP = 320            # routed slots per (core, category)
P_ALL = NCAT * CAP   # 5120 padded routed points per core
U = NCAT * SEG       # 96 stacked logit rows
MCH = NCH // 128     # 32 channel chunks
N_GLOBAL = NCORES * NPTS
BN_EPS = 1e-5
LEAK = 0.2
CHK = 512            # head/softmax column chunk
NCHK = P_ALL // CHK  # 10
OUTW = 50

f32 = mybir.dt.float32
bf16 = mybir.dt.bfloat16
AF = mybir.ActivationFunctionType
ALU = mybir.AluOpType


# CoreSim does not implement the Prelu activation; flip this off (and clear
# _get_program's cache) to build a sim-compatible program for debugging.
USE_PRELU = True


class _Bacc(bacc.Bacc):
    """Keep only natural_log_exp_and_others (exp, ln, parametric_relu,
    copy) so the ACT engine loads exactly one table, once."""

    def insert_act_table_loads(self):
        import bass_rust as _br
        from concourse.hw_specs import get_activation_tables
        has_activation = any(
            isinstance(i, mybir.InstActivation)
            for b in self.main_func.blocks
            for i in b.instructions
        )
        if not has_activation:
            return
        keep = ("natural_log_exp_and_others",)
        tables = [
            (name, funcs if name in keep else set())
            for name, funcs in get_activation_tables(self.m.arch).items()
        ]
        _br.insert_act_table_loads(self, tables)


def build_program():
    nc = _Bacc()

    fnat_d = nc.dram_tensor("fnat", [128, MCH, KF], bf16, kind="ExternalInput")
    fgT_d = nc.dram_tensor("fgT", [128, 2, P_ALL], bf16, kind="ExternalInput")
    w1_d = nc.dram_tensor("w1", [128, 2, NCH], bf16, kind="ExternalInput")
    wnat_d = nc.dram_tensor("wnat", [128, MCH, KF], bf16, kind="ExternalInput")
    wcb_d = nc.dram_tensor("wcb", [128, 2, U], bf16, kind="ExternalInput")
    gam_d = nc.dram_tensor("gamma_t", [128, MCH], f32, kind="ExternalInput")
    bet_d = nc.dram_tensor("beta_t", [128, MCH], f32, kind="ExternalInput")
    r96_d = nc.dram_tensor("r96", [U, U], bf16, kind="ExternalInput")
    bias_d = nc.dram_tensor("bias96", [U, 1], f32, kind="ExternalInput")
    out_d = nc.dram_tensor("out", [U, P_ALL], f32, kind="ExternalOutput")
    stats_in_d = nc.dram_tensor("stats_in", [128, 64], f32)
    stats_out_d = nc.dram_tensor("stats_out", [128, 64], f32, addr_space="Shared")
    warm_in_d = nc.dram_tensor("warm_in", [1, 8], f32)
    warm_out_d = nc.dram_tensor("warm_out", [1, 8], f32, addr_space="Shared")

    with ExitStack() as ctx:
        tc = ctx.enter_context(tile.TileContext(nc))
        big = ctx.enter_context(tc.tile_pool(name="big", bufs=1))
        consts = ctx.enter_context(tc.tile_pool(name="consts", bufs=1))
        stat = ctx.enter_context(tc.tile_pool(name="stat", bufs=1))
        scrp = ctx.enter_context(tc.tile_pool(name="scrp", bufs=2))
        work = ctx.enter_context(tc.tile_pool(name="work", bufs=3))
        psC = ctx.enter_context(tc.tile_pool(name="psC", bufs=1, space="PSUM"))
        psA = ctx.enter_context(tc.tile_pool(name="psA", bufs=2, space="PSUM"))
        psH = ctx.enter_context(tc.tile_pool(name="psH", bufs=2, space="PSUM"))
        psS = ctx.enter_context(tc.tile_pool(name="psS", bufs=1, space="PSUM"))
        psX = ctx.enter_context(tc.tile_pool(name="psX", bufs=1, space="PSUM"))

        # Warm-up collective: pays the cross-core startup-barrier/skew wait
        # and the CC ramp-up concurrently with the compute front, so the
        # real stats AllReduce later starts immediately.
        nc.gpsimd.collective_compute(
            "AllReduce",
            ALU.add,
            replica_groups=[list(range(NCORES))],
            ins=[warm_in_d[:]],
            outs=[warm_out_d[:]],
        )

        # ---------------- loads --------------------------------------------
        wcb = consts.tile([128, 2, U], bf16)
        nc.sync.dma_start(out=wcb, in_=wcb_d[:])
        gam = consts.tile([128, MCH], f32)
        nc.sync.dma_start(out=gam, in_=gam_d[:])
        bet = consts.tile([128, MCH], f32)
        nc.sync.dma_start(out=bet, in_=bet_d[:])
        r96 = consts.tile([U, U], bf16)
        nc.sync.dma_start(out=r96, in_=r96_d[:])
        bias96 = consts.tile([U, 1], f32)
        nc.sync.dma_start(out=bias96, in_=bias_d[:])

        # order by first use: fnat (C) -> w1 (D^T) -> wnat (sumsq reduce)
        # -> fgT (s, routed x1)
        fnat = big.tile([128, MCH, KF], bf16)
        for q in range(4):
            nc.sync.dma_start(
                out=fnat[:, q * 8:(q + 1) * 8, :],
                in_=fnat_d[:, q * 8:(q + 1) * 8, :],
            )
        w1 = big.tile([128, 2, NCH], bf16)
        for q in range(4):
            nc.sync.dma_start(
                out=w1[:, :, q * 1024:(q + 1) * 1024],
                in_=w1_d[:, :, q * 1024:(q + 1) * 1024],
            )
        wnat = big.tile([128, MCH, KF], bf16)
        nc.sync.dma_start(out=wnat, in_=wnat_d[:])
        fgT = big.tile([128, 2, P_ALL], bf16)
        for q in range(4):
            nc.sync.dma_start(
                out=fgT[:, :, q * 4 * CAP:(q + 1) * 4 * CAP],
                in_=fgT_d[:, :, q * 4 * CAP:(q + 1) * 4 * CAP],
            )

        eps_t = consts.tile([128, 1], f32)
        nc.vector.memset(eps_t, BN_EPS)

        # ------------- s = sum_n F[n, :] over this core's routed points ----
        # partial-reduce per fgT DMA chunk so s is ready as the last chunk
        # lands rather than serializing a full-tile pass after it
        s_p = stat.tile([128, 4, 2], f32)
        for q in range(4):
            nc.vector.tensor_reduce(
                out=s_p[:, q, :],
                in_=fgT[:, :, q * 4 * CAP:(q + 1) * 4 * CAP],
                axis=mybir.AxisListType.X, op=ALU.add,
            )
        s_f = stat.tile([128, 2], f32)
        nc.vector.tensor_reduce(
            out=s_f, in_=s_p.rearrange("p q ki -> p ki q"),
            axis=mybir.AxisListType.X, op=ALU.add,
        )
        s_bf = stat.tile([128, 2], bf16)
        nc.vector.tensor_copy(out=s_bf, in_=s_f)

        # ------------- C = F^T F (interleaved kc so it starts on the first
        # fnat DMA chunk) ---------------------------------------------------
        pc0 = psC.tile([128, KF], f32, tag="pc0")
        pc1 = psC.tile([128, KF], f32, tag="pc1")
        for i in range(MCH):
            for kc in range(2):
                nc.tensor.matmul(
                    pc0 if kc == 0 else pc1,
                    lhsT=fnat[:, i, kc * 128:(kc + 1) * 128],
                    rhs=fnat[:, i, :],
                    start=(i == 0),
                    stop=(i == MCH - 1),
                )
        C_sb = stat.tile([128, 2, KF], bf16)
        nc.vector.tensor_copy(out=C_sb[:, 0, :], in_=pc0)
        nc.vector.tensor_copy(out=C_sb[:, 1, :], in_=pc1)

        # stats tile: [0:32] sumx1 partial, [32:64] sumsq partial
        stats_sb = stat.tile([128, 64], f32)

        # ------------- D^T = W1^T C with channels on partitions; multiply-
        # reduce against W1nat gives sumsq directly in stats layout ---------
        for m in range(MCH):
            pdt = psA.tile([128, CAP], f32, tag="x1")
            pd = pdt[:, 0:KF]
            for ll in range(2):
                nc.tensor.matmul(
                    pd,
                    lhsT=w1[:, ll, m * 128:(m + 1) * 128],
                    rhs=C_sb[:, ll, :],
                    start=(ll == 0),
                    stop=(ll == 1),
                )
            scr = scrp.tile([128, KF], bf16, tag="sq")
            nc.vector.tensor_mul(out=scr, in0=pd, in1=wnat[:, m, :])
            nc.vector.tensor_reduce(
                out=stats_sb[:, 32 + m:32 + m + 1], in_=scr,
                axis=mybir.AxisListType.X, op=ALU.add,
            )

        # sumx1 partial: stats column m = W1 chunk^T s (baseline p_sx shape)
        p_sx = psX.tile([128, MCH], f32, tag="psx")
        for m in range(MCH):
            for kc in range(2):
                nc.tensor.matmul(
                    p_sx[:, m:m + 1],
                    lhsT=w1[:, kc, m * 128:(m + 1) * 128],
                    rhs=s_bf[:, kc:kc + 1],
                    start=(kc == 0),
                    stop=(kc == 1),
                )
        nc.vector.tensor_copy(out=stats_sb[:, 0:MCH], in_=p_sx)

        wr = nc.sync.dma_start(out=stats_in_d[:], in_=stats_sb)
        cc = nc.gpsimd.collective_compute(
            "AllReduce",
            ALU.add,
            replica_groups=[list(range(NCORES))],
            ins=[stats_in_d[:]],
            outs=[stats_out_d[:]],
        )
        add_dep_helper(cc.ins, wr.ins, reason="stats written before allreduce")

        # ------------- routed x1 (runs under the AllReduce) -----------------
        # chunk (c, kc) = channels c*256 + kc*128 + p
        x1raw = big.tile([128, 2, P_ALL], bf16)   # [p, kc, c*CAP+s]
        for c in range(NCAT):
            for kc in range(2):
                px = psA.tile([128, CAP], f32, tag="x1")
                for ki in range(2):
                    nc.tensor.matmul(
                        px,
                        lhsT=w1[:, ki, c * 256 + kc * 128:c * 256 + kc * 128 + 128],
                        rhs=fgT[:, ki, c * CAP:(c + 1) * CAP],
                        start=(ki == 0),
                        stop=(ki == 1),
                    )
                if (2 * c + kc) % 2 == 0:
                    nc.vector.tensor_copy(
                        out=x1raw[:, kc, c * CAP:(c + 1) * CAP], in_=px
                    )
                else:
                    nc.scalar.copy(
                        out=x1raw[:, kc, c * CAP:(c + 1) * CAP], in_=px
                    )

        stats_g = stat.tile([128, 64], f32)
        rd = nc.sync.dma_start(out=stats_g, in_=stats_out_d[:])
        add_dep_helper(rd.ins, cc.ins, reason="allreduce before readback")

        # ---------------- a, b (BN affine) ----------------------------------
        mu = stat.tile([128, MCH], f32)
        nc.vector.tensor_scalar(
            out=mu, in0=stats_g[:, 0:MCH], scalar1=1.0 / N_GLOBAL, scalar2=None,
            op0=ALU.mult,
        )
        var = stat.tile([128, MCH], f32)
        nc.vector.tensor_scalar(
            out=var, in0=stats_g[:, MCH:2 * MCH], scalar1=1.0 / N_GLOBAL,
            scalar2=None, op0=ALU.mult,
        )
        mu2 = stat.tile([128, MCH], f32)
        nc.vector.tensor_mul(out=mu2, in0=mu, in1=mu)
        nc.vector.tensor_sub(out=var, in0=var, in1=mu2)
        # rstd = exp(-0.5 * ln(var + eps)) -- stays on the single ACT table
        lnv = stat.tile([128, MCH], f32)
        nc.scalar.activation(out=lnv, in_=var, func=AF.Ln, bias=eps_t, scale=1.0)
        rstd = stat.tile([128, MCH], f32)
        nc.scalar.activation(out=rstd, in_=lnv, func=AF.Exp, scale=-0.5)
        a_t = stat.tile([128, MCH], f32)
        nc.vector.tensor_mul(out=a_t, in0=gam, in1=rstd)
        b_t = stat.tile([128, MCH], f32)
        nc.vector.tensor_mul(out=b_t, in0=mu, in1=a_t)
        nc.vector.tensor_sub(out=b_t, in0=bet, in1=b_t)

        # ---------------- normalize: x2 = LeakyReLU(a*x1 + b) ----------------
        x2all = big.tile([128, 2, P_ALL], bf16)
        for c in range(NCAT):
            for kc in range(2):
                m = 2 * c + kc
                src = x1raw[:, kc, c * CAP:(c + 1) * CAP]
                dst = x2all[:, kc, c * CAP:(c + 1) * CAP]
                if USE_PRELU and m % 16 < 10:
                    nc.scalar.activation(
                        out=dst, in_=src, func=AF.Prelu,
                        bias=b_t[:, m:m + 1], scale=a_t[:, m:m + 1], alpha=LEAK,
                    )
                else:
                    y = scrp.tile([128, CAP], bf16, tag="y")
                    nc.vector.tensor_scalar(
                        out=y, in0=src, scalar1=a_t[:, m:m + 1],
                        scalar2=b_t[:, m:m + 1], op0=ALU.mult, op1=ALU.add,
                    )
                    t02 = scrp.tile([128, CAP], bf16, tag="t02")
                    nc.vector.tensor_scalar_mul(out=t02, in0=y, scalar1=LEAK)
                    nc.vector.tensor_tensor(out=dst, in0=y, in1=t02, op=ALU.max)

        # ------------- head + grouped log-softmax, 512-col chunks ------------
        for t in range(NCHK):
            lo = t * CHK
            pf = psH.tile([U, CHK], f32, tag="pf")
            for kc in range(2):
                nc.tensor.matmul(
                    pf,
                    lhsT=wcb[:, kc, :],
                    rhs=x2all[:, kc, lo:lo + CHK],
                    start=(kc == 0),
                    stop=(kc == 1),
                )
            e = work.tile([U, CHK], bf16, tag="e")
            nc.scalar.activation(
                out=e, in_=pf, func=AF.Exp, bias=bias96, scale=1.0
            )
            # R96 (block-diag ones) sums each 6-row group AND broadcasts the
            # sum back to all 6 rows in one matmul
            ps = psS.tile([U, CHK], f32, tag="ps")
            nc.tensor.matmul(ps, lhsT=r96, rhs=e, start=True, stop=True)
            lnS = work.tile([U, CHK], bf16, tag="lnS")
            nc.scalar.activation(out=lnS, in_=ps, func=AF.Ln)
            tb = work.tile([U, CHK], f32, tag="tb")
            nc.vector.tensor_scalar(
                out=tb, in0=pf, scalar1=bias96, scalar2=None, op0=ALU.add,
            )
            lsm = work.tile([U, CHK], f32, tag="lsm")
            nc.vector.tensor_tensor(out=lsm, in0=tb, in1=lnS, op=ALU.subtract)
            nc.sync.dma_start(out=out_d[:, lo:lo + CHK], in_=lsm)

    if not nc.is_finalized():
        nc.finalize()
    return nc


@functools.lru_cache(maxsize=1)
def _get_program():
    return build_program()


def _host_prep(features, W1, gamma, beta, Wc, bias, cats, shifts, seg_lens):
    features = np.ascontiguousarray(np.asarray(features, dtype=np.float32))
    W1 = np.ascontiguousarray(np.asarray(W1, dtype=np.float32))
    gamma = np.asarray(gamma, dtype=np.float32)
    beta = np.asarray(beta, dtype=np.float32)
    Wc = np.asarray(Wc, dtype=np.float32)
    bias = np.asarray(bias, dtype=np.float32)
    cats = np.asarray(cats)

    # route: global sort by category, split each category across the 8 cores
    order = np.argsort(cats, kind="stable")
    counts = np.bincount(cats, minlength=NCAT)
    starts = np.concatenate([[0], np.cumsum(counts)[:-1]])
    gidx = [[None] * NCAT for _ in range(NCORES)]
    for c in range(NCAT):
        pts = order[starts[c]:starts[c] + counts[c]]
        splits = np.array_split(pts, NCORES)
        for ci in range(NCORES):
            assert len(splits[ci]) <= CAP, (
                f"category {c} count {counts[c]} exceeds capacity"
            )
            gidx[ci][c] = splits[ci]

    # wcb[p, kc, 6c+j] = Wc[c, kc*128+p, j]
    wcb = np.zeros((128, 2, U), np.float32)
    for c in range(NCAT):
        for kc in range(2):
            wcb[:, kc, 6 * c:6 * c + 6] = Wc[c, kc * 128:(kc + 1) * 128, :]

    r96 = np.zeros((U, U), np.float32)
    for c in range(NCAT):
        r96[6 * c:6 * c + 6, 6 * c:6 * c + 6] = 1.0

    common = {
        "w1": np.ascontiguousarray(
            W1.reshape(2, 128, NCH).transpose(1, 0, 2)
        ).astype(BF),
        "wnat": np.ascontiguousarray(
            W1.T.reshape(MCH, 128, KF).transpose(1, 0, 2)
        ).astype(BF),
        "wcb": wcb.astype(BF),
        "gamma_t": np.ascontiguousarray(gamma.reshape(MCH, 128).T),
        "beta_t": np.ascontiguousarray(beta.reshape(MCH, 128).T),
        "r96": r96.astype(BF),
        "bias96": np.tile(bias, NCAT).astype(np.float32).reshape(U, 1),
    }

    fT = features.T.astype(BF)  # [256, N]
    in_maps = []
    for ci in range(NCORES):
        fc = features[ci * NPTS:(ci + 1) * NPTS]
        fg = np.zeros((128, 2, P_ALL), BF)
        for c in range(NCAT):
            g = gidx[ci][c]
            blk = fT[:, g].reshape(2, 128, len(g))
            fg[:, :, c * CAP:c * CAP + len(g)] = blk.transpose(1, 0, 2)
        m = dict(common)
        m["fgT"] = fg
        m["fnat"] = np.ascontiguousarray(
            fc.reshape(MCH, 128, KF).transpose(1, 0, 2)
        ).astype(BF)
        in_maps.append(m)
    return in_maps, gidx


def _assemble(results, gidx, shifts, seg_lens):
    shifts = np.asarray(shifts).astype(np.int64)
    seg_lens = np.asarray(seg_lens).astype(np.int64)
    out = np.zeros((NCORES * NPTS, OUTW), np.float32)
    for ci in range(NCORES):
        lsm = results[ci]["out"]  # [U, P_ALL] f32
        for c in range(NCAT):
            g = gidx[ci][c]
            if len(g) == 0:
                continue
            ln = int(seg_lens[c])
            sh = int(shifts[c])
            blk = lsm[6 * c:6 * c + ln, c * CAP:c * CAP + len(g)]
            out[np.ix_(g, np.arange(sh, sh + ln))] = blk.T
    return out


def kernel(**inputs):
    in_maps, gidx = _host_prep(
        inputs["features"], inputs["W1"], inputs["gamma"], inputs["beta"],
        inputs["Wc"], inputs["bias"], inputs["cats"], inputs["shifts"],
        inputs["seg_lens"],
    )
    nc = _get_program()
    res = run_bass_kernel_spmd(nc, in_maps, core_ids=list(range(NCORES)))
    return _assemble(res.results, gidx, inputs["shifts"], inputs["seg_lens"])


# used by test.py for profiling runs
def kernel_traced(**inputs):
    in_maps, gidx = _host_prep(
        inputs["features"], inputs["W1"], inputs["gamma"], inputs["beta"],
        inputs["Wc"], inputs["bias"], inputs["cats"], inputs["shifts"],
        inputs["seg_lens"],
    )
    nc = _get_program()
    res = run_bass_kernel_spmd(
        nc, in_maps, core_ids=list(range(NCORES)), trace=True
    )
    out = _assemble(res.results, gidx, inputs["shifts"], inputs["seg_lens"])
    return out, res
